# revision 33
# baseline (speedup 1.0000x reference)
"""Single-head attention (SEQ=8192, D_MODEL=2048, D_K=128) on 8 TRN2 NeuronCores.

Sharding: sequence-parallel. Each core owns 1024 query rows. QKV projections
computed on the local shard; K^T and V all-gathered across the 8 cores
(d_k=128 lives on the partition dim, so gathered blocks DMA straight into
matmul operand layouts with no device-side transposes). Attention runs in
S^T layout ([key, query] tiles): exp on the scalar engine, P@V on the tensor
engine (software-pipelined one block behind the scores), and the softmax
denominator accumulated on the vector engine.
"""
import os

import numpy as np

import concourse.bacc as bacc
import concourse.tile as tile
from concourse import mybir
from concourse.bass_utils import run_bass_kernel_spmd

N_CORES = 8
SEQ = 8192
DM = 2048
DK = 128
SL = SEQ // N_CORES          # 1024 local rows
NMC = DM // 128              # 16 contraction chunks for projections
NKB = SEQ // 128             # 64 key blocks
NLB = SL // 128              # 8 key blocks per gathered core-block
SCALE = float(np.sqrt(DK))

F32 = mybir.dt.float32


def _build(mm_dt, cut="full"):
    nc = bacc.Bacc(
        "TRN2",
        target_bir_lowering=False,
        debug=False,
        num_devices=N_CORES,
    )

    xT = nc.dram_tensor("xT", [DM, SL], mm_dt, kind="ExternalInput")
    wqT = nc.dram_tensor("wqT", [DM, DK], mm_dt, kind="ExternalInput")
    wkT = nc.dram_tensor("wkT", [DM, DK], mm_dt, kind="ExternalInput")
    wvT = nc.dram_tensor("wvT", [DM, DK], mm_dt, kind="ExternalInput")
    ones_d = nc.dram_tensor("ones_d", [128, 128], mm_dt, kind="ExternalInput")
    ident_d = nc.dram_tensor("ident_d", [128, 128], mm_dt, kind="ExternalInput")
    b_pack = nc.dram_tensor("b_pack", [DK, 3], F32, kind="ExternalInput")
    out = nc.dram_tensor("out", [SL, DK], F32, kind="ExternalOutput")

    CP = mybir.ActivationFunctionType.Copy
    ID = mybir.ActivationFunctionType.Identity

    with tile.TileContext(nc) as tc:
        with (
            tc.tile_pool(name="const", bufs=1) as const_pool,
            tc.tile_pool(name="proj", bufs=1) as proj_pool,
            tc.tile_pool(name="kv", bufs=1) as kv_pool,
            tc.tile_pool(name="w", bufs=1) as w_pool,
            tc.tile_pool(name="vs", bufs=4) as vs_pool,
            tc.tile_pool(name="pt", bufs=4) as pt_pool,
            tc.tile_pool(name="fin", bufs=1) as fin_pool,
            tc.tile_pool(name="dram", bufs=1, space="DRAM") as dram_pool,
        ):
            # ---- constants (loads emitted after the first x group) ----
            ident = const_pool.tile([128, 128], mm_dt)
            onesf = const_pool.tile([128, 128], F32)
            b_sb = const_pool.tile([DK, 3], F32)
            bq_sb, bk_sb, bv_sb = (b_sb[:, 0:1], b_sb[:, 1:2], b_sb[:, 2:3])

            # ---- inputs loaded per-chunk (first matmuls start after ~2 DMAs)
            wk_t = w_pool.tile([128, NMC, DK], mm_dt)
            wq_t = w_pool.tile([128, NMC, DK], mm_dt)
            wv_t = w_pool.tile([128, NMC, DK], mm_dt)
            half = (NMC // 2) * 128
            nc.sync.dma_start(
                wk_t[:, :NMC // 2, :],
                wkT[:half, :].rearrange("(c p) d -> p c d", p=128))
            wk_sb = [wk_t[:, i, :] for i in range(NMC)]
            wq_sb = [wq_t[:, i, :] for i in range(NMC)]
            wv_sb = [wv_t[:, i, :] for i in range(NMC)]
            XG = 2                       # x chunks per load group
            x_g = [w_pool.tile([128, XG, SL], mm_dt, name=f"xg{g}")
                   for g in range(NMC // XG)]
            x_sb = [x_g[i // XG][:, i % XG, :] for i in range(NMC)]
            for g in range(NMC // XG):
                if g == 1:
                    nc.sync.dma_start(
                        wk_t[:, NMC // 2:, :],
                        wkT[half:, :].rearrange("(c p) d -> p c d", p=128))
                    nc.sync.dma_start(b_sb[:], b_pack[:])
                    nc.sync.dma_start(ident[:], ident_d[:])
                    nc.sync.dma_start(onesf[:], ones_d[:].bitcast(F32))
                if g == 2:
                    nc.sync.dma_start(
                        wq_t[:], wqT.rearrange("(c p) d -> p c d", p=128))
                if g == 3:
                    nc.sync.dma_start(
                        wv_t[:], wvT.rearrange("(c p) d -> p c d", p=128))
                nc.sync.dma_start(
                    x_g[g][:],
                    xT[g * XG * 128:(g + 1) * XG * 128, :].rearrange(
                        "(c p) s -> p c s", p=128))

            qt_sb = proj_pool.tile([128, SL], mm_dt)
            kt_loc = proj_pool.tile([128, SL], mm_dt)
            vt_loc = proj_pool.tile([128, SL], mm_dt)
            kt_dram = dram_pool.tile([128, SL], mm_dt)
            vc_dram = dram_pool.tile([SL, DK], mm_dt)

            ktg_dram = dram_pool.tile(
                [N_CORES, 128, SL], mm_dt, addr_space="Shared")
            vg_dram = dram_pool.tile(
                [N_CORES, SL, DK], mm_dt, addr_space="Shared")
            groups = [list(range(N_CORES))]
            if cut == "proj":
                groups = None  # sections below are skipped
            skip_cc = os.environ.get("KCC", "") == "skip"

            KTG = 4                      # gathered core-blocks per staging DMA
            kt_g = [kv_pool.tile([128, KTG * SL], mm_dt, name=f"ktg{g}")
                    for g in range(N_CORES // KTG)]
            kt_b = [kt_g[b // KTG][:, (b % KTG) * SL:(b % KTG + 1) * SL]
                    for b in range(N_CORES)]
            v_g = [kv_pool.tile([128, KTG * NLB, DK], mm_dt, name=f"vg{g}")
                   for g in range(N_CORES // KTG)]
            v_b = [v_g[b // KTG][:, (b % KTG) * NLB:(b % KTG + 1) * NLB, :]
                   for b in range(N_CORES)]

            # ---- phase A: projections, ordered K -> Q -> V so the K gather
            # (which gates the attention MM1s) launches as early as possible.
            with tc.tile_pool(name="ps_proj", bufs=2, space="PSUM") as ps_proj:
                kt_ps = ps_proj.tile([128, SL], F32, tag="proj")
                for i in range(NMC):
                    for h in range(2):
                        hs = slice(h * 512, (h + 1) * 512)
                        nc.tensor.matmul(kt_ps[:, hs], wk_sb[i],
                                         x_sb[i][:, hs],
                                         start=(i == 0), stop=(i == NMC - 1))
                nc.scalar.activation(kt_loc[:], kt_ps[:], ID, bias=bk_sb[:])
                nc.sync.dma_start(kt_dram[:], kt_loc[:])
                if groups and not skip_cc:
                    nc.gpsimd.collective_compute(
                        "AllGather", mybir.AluOpType.bypass,
                        replica_groups=groups,
                        ins=[kt_dram.opt()], outs=[ktg_dram.opt()],
                    )
                # Q projection (overlaps the K gather)
                qt_ps = ps_proj.tile([128, SL], F32, tag="proj")
                for i in range(NMC):
                    for h in range(2):
                        hs = slice(h * 512, (h + 1) * 512)
                        nc.tensor.matmul(qt_ps[:, hs], wq_sb[i],
                                         x_sb[i][:, hs],
                                         start=(i == 0), stop=(i == NMC - 1))
                nc.scalar.activation(qt_sb[:], qt_ps[:], ID, bias=bq_sb[:])
                for b in range(N_CORES if groups else 0):
                    nc.sync.dma_start(
                        kt_g[b // KTG][:, (b % KTG) * SL:(b % KTG + 1) * SL],
                        ktg_dram[b])

                # V projection + PE transpose to natural layout
                vt_ps = ps_proj.tile([128, SL], F32, tag="proj")
                for i in range(NMC):
                    for h in range(2):
                        hs = slice(h * 512, (h + 1) * 512)
                        nc.tensor.matmul(vt_ps[:, hs], wv_sb[i],
                                         x_sb[i][:, hs],
                                         start=(i == 0), stop=(i == NMC - 1))
                nc.scalar.activation(vt_loc[:], vt_ps[:], ID, bias=bv_sb[:])
                vsb_all = vs_pool.tile([128, NLB, 128], mm_dt, tag="vsb",
                                       bufs=1)
                with tc.tile_pool(name="ps_vtr", bufs=2, space="PSUM") as ps_vtr:
                    for t in range(NLB):
                        vtr = ps_vtr.tile([128, 128], mm_dt, tag="vtr")
                        nc.tensor.transpose(
                            vtr[:], vt_loc[:, t * 128:(t + 1) * 128], ident[:])
                        nc.scalar.copy(vsb_all[:, t, :], vtr[:])
                    nc.sync.dma_start(
                        vc_dram.rearrange("(t p) d -> p t d", p=128),
                        vsb_all[:])
                if groups and not skip_cc:
                    nc.gpsimd.collective_compute(
                        "AllGather", mybir.AluOpType.bypass,
                        replica_groups=groups,
                        ins=[vc_dram.opt()], outs=[vg_dram.opt()],
                    )
                for b in range(N_CORES if groups else 0):
                    nc.sync.dma_start(
                        v_g[b // KTG][:, (b % KTG) * NLB:(b % KTG + 1) * NLB, :],
                        vg_dram[b].rearrange("(t p) d -> p t d", p=128))

            if cut == "proj":
                nc.sync.dma_start(out[0:128, :], qt_sb[:, 0:128].bitcast(F32))
            if cut == "gather":
                nc.sync.dma_start(out[0:128, :], v_b[0][:, 0, :].bitcast(F32))

            # ---- phase B: attention over all 1024 queries at once,
            # one loop over the 64 key blocks, software-pipelined ----
            NJ = NKB if cut == "full" else 0
            with (
                tc.tile_pool(name="ps_st", bufs=3, space="PSUM") as ps_st,
                tc.tile_pool(name="ps_o", bufs=1, space="PSUM") as ps_o,
            ):
                o_ps = ps_o.tile([128, SL], F32, tag="o")
                acc = fin_pool.tile([128, SL], F32, tag="acc")
                acc_g = fin_pool.tile([128, SL], F32, tag="acc_g")
                pts = {}
                n_d = n_g = 0
                for j in range(NJ + 1):
                    if j < NJ:
                        jb, jo = divmod(j, NLB)
                        st_ps = ps_st.tile([128, SL], F32, tag="st")
                        for u in range(2):
                            nc.tensor.matmul(
                                st_ps[:, u * 512:(u + 1) * 512],
                                kt_b[jb][:, jo * 128:(jo + 1) * 128],
                                qt_sb[:, u * 512:(u + 1) * 512],
                                start=True, stop=True,
                            )
                        pt = pt_pool.tile([128, SL], mm_dt, tag="pt")
                        nc.scalar.activation(
                            pt[:], st_ps[:], mybir.ActivationFunctionType.Exp)
                        pts[j] = pt
                    if j > 0:
                        pt = pts.pop(j - 1)
                        jb, jo = divmod(j - 1, NLB)
                        for u in range(2):
                            nc.tensor.matmul(
                                o_ps[:, u * 512:(u + 1) * 512],
                                v_b[jb][:, jo, :],
                                pt[:, u * 512:(u + 1) * 512],
                                start=(j - 1 == 0), stop=(j - 1 == NKB - 1))
                        if (j - 1) % 3 == 2:
                            if n_g == 0:
                                nc.gpsimd.tensor_copy(
                                    acc_g[:], pt[:].bitcast(F32))
                            else:
                                nc.gpsimd.tensor_add(
                                    acc_g[:], acc_g[:], pt[:].bitcast(F32))
                            n_g += 1
                        else:
                            if n_d == 0:
                                nc.vector.tensor_copy(acc[:], pt[:].bitcast(F32))
                            else:
                                nc.vector.tensor_add(
                                    acc[:], acc[:], pt[:].bitcast(F32))
                            n_d += 1

                if NJ:
                    # partition-reduce the exp accumulator, then normalize O^T
                    nc.vector.tensor_add(acc[:], acc[:], acc_g[:])
                    cs_ps = ps_st.tile([128, SL], F32, tag="st")
                    for u in range(2):
                        us = slice(u * 512, (u + 1) * 512)
                        nc.tensor.matmul(cs_ps[:, us], onesf[:], acc[:, us],
                                         start=True, stop=True)
                    o_sb = fin_pool.tile([128, SL], F32, tag="o_sb")
                    rcs = fin_pool.tile([128, SL], F32, tag="rcs")
                    oo_all = fin_pool.tile([128, NLB, DK], F32, tag="oo")
                    for u in range(2):
                        us = slice(u * 512, (u + 1) * 512)
                        nc.vector.reciprocal(rcs[:, us], cs_ps[:, us])
                        nc.vector.tensor_mul(o_sb[:, us], o_ps[:, us], rcs[:, us])
                        for t in range(u * 4, u * 4 + 4):
                            otr = ps_st.tile([128, 128], F32, tag="st")
                            nc.tensor.transpose(
                                otr[:], o_sb[:, t * 128:(t + 1) * 128],
                                ident[:].bitcast(F32))
                            nc.vector.tensor_copy(oo_all[:, t, :], otr[:])
                    nc.sync.dma_start(
                        out.rearrange("(t p) d -> p t d", p=128), oo_all[:])

    nc.compile()
    return nc


_NC_CACHE = {}


def _get_nc(mm_dt):
    cut = os.environ.get("KCUT", "full")
    key = (str(mm_dt), cut, os.environ.get("KCC", ""))
    if key not in _NC_CACHE:
        _NC_CACHE[key] = _build(mm_dt, cut)
    return _NC_CACHE[key]


def _run(inputs, trace=False, mm_dt=None, **spmd_kwargs):
    if mm_dt is None:
        mm_dt = (mybir.dt.float32r
                 if os.environ.get("KDT", "f32r") == "f32r" else F32)
    x = np.asarray(inputs["x"], dtype=np.float32)
    Wq = np.asarray(inputs["Wq"], dtype=np.float32)
    Wk = np.asarray(inputs["Wk"], dtype=np.float32)
    Wv = np.asarray(inputs["Wv"], dtype=np.float32)
    bq = np.asarray(inputs["bq"], dtype=np.float32)
    bk = np.asarray(inputs["bk"], dtype=np.float32)
    bv = np.asarray(inputs["bv"], dtype=np.float32)

    wqT = np.ascontiguousarray((Wq / SCALE).T)
    wkT = np.ascontiguousarray(Wk.T)
    wvT = np.ascontiguousarray(Wv.T)
    shared = {
        "wqT": wqT, "wkT": wkT, "wvT": wvT,
        "b_pack": np.ascontiguousarray(
            np.stack([bq / SCALE, bk, bv], axis=1).astype(np.float32)),
        "ones_d": np.ones((128, 128), dtype=np.float32),
        "ident_d": np.eye(128, dtype=np.float32),
    }
    in_maps = []
    for c in range(N_CORES):
        xT_c = np.ascontiguousarray(x[c * SL:(c + 1) * SL].T)
        in_maps.append({"xT": xT_c, **shared})

    nc = _get_nc(mm_dt)
    res = run_bass_kernel_spmd(
        nc, in_maps, core_ids=list(range(N_CORES)), trace=trace, **spmd_kwargs)
    full = np.concatenate(
        [res.results[c]["out"] for c in range(N_CORES)], axis=0)
    return full, res


def kernel(**inputs):
    out, _ = _run(inputs)
    return out


# revision 36
# speedup vs baseline: 762.2505x; 762.2505x over previous
"""Single-head attention (SEQ=8192, D_MODEL=2048, D_K=128) on 8 TRN2 NeuronCores.

Sharding: sequence-parallel. Each core owns 1024 query rows. QKV projections
computed on the local shard; K^T and V all-gathered across the 8 cores
(d_k=128 lives on the partition dim, so gathered blocks DMA straight into
matmul operand layouts with no device-side transposes). Attention runs in
S^T layout ([key, query] tiles): exp on the scalar engine, P@V on the tensor
engine (software-pipelined one block behind the scores), and the softmax
denominator accumulated on the vector engine.
"""
import os

import numpy as np

import concourse.bacc as bacc
import concourse.tile as tile
from concourse import mybir
from concourse.bass_utils import run_bass_kernel_spmd

N_CORES = 8
SEQ = 8192
DM = 2048
DK = 128
SL = SEQ // N_CORES          # 1024 local rows
NMC = DM // 128              # 16 contraction chunks for projections
NKB = SEQ // 128             # 64 key blocks
NLB = SL // 128              # 8 key blocks per gathered core-block
SCALE = float(np.sqrt(DK))

F32 = mybir.dt.float32


def _build(mm_dt, cut="full"):
    nc = bacc.Bacc(
        "TRN2",
        target_bir_lowering=False,
        debug=False,
        num_devices=N_CORES,
    )

    xT = nc.dram_tensor("xT", [DM, SL], mm_dt, kind="ExternalInput")
    wqT = nc.dram_tensor("wqT", [DM, DK], mm_dt, kind="ExternalInput")
    wkT = nc.dram_tensor("wkT", [DM, DK], mm_dt, kind="ExternalInput")
    wvT = nc.dram_tensor("wvT", [DM, DK], mm_dt, kind="ExternalInput")
    ones_d = nc.dram_tensor("ones_d", [128, 128], mm_dt, kind="ExternalInput")
    ident_d = nc.dram_tensor("ident_d", [128, 128], mm_dt, kind="ExternalInput")
    b_pack = nc.dram_tensor("b_pack", [DK, 3], F32, kind="ExternalInput")
    out = nc.dram_tensor("out", [SL, DK], F32, kind="ExternalOutput")

    CP = mybir.ActivationFunctionType.Copy
    ID = mybir.ActivationFunctionType.Identity

    with tile.TileContext(nc) as tc:
        with (
            tc.tile_pool(name="const", bufs=1) as const_pool,
            tc.tile_pool(name="proj", bufs=1) as proj_pool,
            tc.tile_pool(name="kv", bufs=1) as kv_pool,
            tc.tile_pool(name="w", bufs=1) as w_pool,
            tc.tile_pool(name="vs", bufs=4) as vs_pool,
            tc.tile_pool(name="pt", bufs=4) as pt_pool,
            tc.tile_pool(name="fin", bufs=1) as fin_pool,
            tc.tile_pool(name="dram", bufs=1, space="DRAM") as dram_pool,
        ):
            # ---- constants (loads emitted after the first x group) ----
            ident = const_pool.tile([128, 128], mm_dt)
            onesf = const_pool.tile([128, 128], F32)
            b_sb = const_pool.tile([DK, 3], F32)
            bq_sb, bk_sb, bv_sb = (b_sb[:, 0:1], b_sb[:, 1:2], b_sb[:, 2:3])

            # ---- inputs loaded per-chunk (first matmuls start after ~2 DMAs)
            wk_t = w_pool.tile([128, NMC, DK], mm_dt)
            wq_t = w_pool.tile([128, NMC, DK], mm_dt)
            wv_t = w_pool.tile([128, NMC, DK], mm_dt)
            half = (NMC // 2) * 128
            nc.sync.dma_start(
                wk_t[:, :NMC // 2, :],
                wkT[:half, :].rearrange("(c p) d -> p c d", p=128))
            wk_sb = [wk_t[:, i, :] for i in range(NMC)]
            wq_sb = [wq_t[:, i, :] for i in range(NMC)]
            wv_sb = [wv_t[:, i, :] for i in range(NMC)]
            # x loaded in column halves (local sequence halves), half 0
            # first, so the K projection for half 0 — and its all-gather —
            # start as early as possible.
            XG = 2                       # x chunks per load group
            NG = NMC // XG
            x_hg = [[w_pool.tile([128, XG, 512], mm_dt, name=f"x{h}g{g}")
                     for g in range(NG)] for h in range(2)]

            def x_half(i, h):
                return x_hg[h][i // XG][:, i % XG, :]

            for h in range(2):
                for g in range(NG):
                    if h == 0 and g == 1:
                        nc.sync.dma_start(
                            wk_t[:, NMC // 2:, :],
                            wkT[half:, :].rearrange("(c p) d -> p c d", p=128))
                        nc.sync.dma_start(b_sb[:], b_pack[:])
                        nc.sync.dma_start(ident[:], ident_d[:])
                        nc.sync.dma_start(onesf[:], ones_d[:].bitcast(F32))
                    if h == 0 and g == 4:
                        nc.sync.dma_start(
                            wq_t[:], wqT.rearrange("(c p) d -> p c d", p=128))
                    if h == 1 and g == 0:
                        nc.sync.dma_start(
                            wv_t[:], wvT.rearrange("(c p) d -> p c d", p=128))
                    nc.sync.dma_start(
                        x_hg[h][g][:],
                        xT[g * XG * 128:(g + 1) * XG * 128,
                           h * 512:(h + 1) * 512].rearrange(
                            "(c p) s -> p c s", p=128))

            qt_sb = proj_pool.tile([128, SL], mm_dt)
            vt_loc = proj_pool.tile([128, SL], mm_dt)
            kth_loc = [proj_pool.tile([128, 512], mm_dt, name=f"ktl{h}")
                       for h in range(2)]
            ktd_h = [dram_pool.tile([128, 512], mm_dt, name=f"ktd{h}")
                     for h in range(2)]
            vc_dram = dram_pool.tile([SL, DK], mm_dt)

            ktg_h = [dram_pool.tile([N_CORES, 128, 512], mm_dt,
                                    addr_space="Shared", name=f"ktgd{h}")
                     for h in range(2)]
            vg_dram = dram_pool.tile(
                [N_CORES, SL, DK], mm_dt, addr_space="Shared")
            groups = [list(range(N_CORES))]
            if cut == "proj":
                groups = None  # sections below are skipped
            skip_cc = os.environ.get("KCC", "") == "skip"

            KTG = 4                      # gathered core-blocks per staging DMA
            # gathered K^T, one tile per sequence half: [128, 8 cores x 512]
            kth = [kv_pool.tile([128, N_CORES * 512], mm_dt, name=f"kth{h}")
                   for h in range(2)]
            v_g = [kv_pool.tile([128, KTG * NLB, DK], mm_dt, name=f"vg{g}")
                   for g in range(N_CORES // KTG)]
            v_b = [v_g[b // KTG][:, (b % KTG) * NLB:(b % KTG + 1) * NLB, :]
                   for b in range(N_CORES)]

            # ---- phase A: projections. K in sequence halves (each half's
            # all-gather launches as soon as that half of x has landed),
            # then Q, then V.
            with tc.tile_pool(name="ps_proj", bufs=2, space="PSUM") as ps_proj:
                for h in range(2):
                    kt_ps_h = ps_proj.tile([128, 512], F32, tag="projh")
                    for i in range(NMC):
                        nc.tensor.matmul(kt_ps_h[:], wk_sb[i], x_half(i, h),
                                         start=(i == 0), stop=(i == NMC - 1))
                    nc.scalar.activation(
                        kth_loc[h][:], kt_ps_h[:], ID, bias=bk_sb[:])
                    nc.sync.dma_start(ktd_h[h][:], kth_loc[h][:])
                    if groups and not skip_cc:
                        nc.gpsimd.collective_compute(
                            "AllGather", mybir.AluOpType.bypass,
                            replica_groups=groups,
                            ins=[ktd_h[h].opt()], outs=[ktg_h[h].opt()],
                        )
                    for b in range(N_CORES if groups else 0):
                        nc.sync.dma_start(
                            kth[h][:, b * 512:(b + 1) * 512], ktg_h[h][b])

                # Q projection (overlaps the K gathers)
                qt_ps = ps_proj.tile([128, SL], F32, tag="proj")
                for i in range(NMC):
                    for h in range(2):
                        hs = slice(h * 512, (h + 1) * 512)
                        nc.tensor.matmul(qt_ps[:, hs], wq_sb[i],
                                         x_half(i, h),
                                         start=(i == 0), stop=(i == NMC - 1))
                nc.scalar.activation(qt_sb[:], qt_ps[:], ID, bias=bq_sb[:])

                # V projection + PE transpose to natural layout
                vt_ps = ps_proj.tile([128, SL], F32, tag="proj")
                for i in range(NMC):
                    for h in range(2):
                        hs = slice(h * 512, (h + 1) * 512)
                        nc.tensor.matmul(vt_ps[:, hs], wv_sb[i],
                                         x_half(i, h),
                                         start=(i == 0), stop=(i == NMC - 1))
                nc.scalar.activation(vt_loc[:], vt_ps[:], ID, bias=bv_sb[:])
                vsb_all = vs_pool.tile([128, NLB, 128], mm_dt, tag="vsb",
                                       bufs=1)
                with tc.tile_pool(name="ps_vtr", bufs=2, space="PSUM") as ps_vtr:
                    for t in range(NLB):
                        vtr = ps_vtr.tile([128, 128], mm_dt, tag="vtr")
                        nc.tensor.transpose(
                            vtr[:], vt_loc[:, t * 128:(t + 1) * 128], ident[:])
                        nc.scalar.copy(vsb_all[:, t, :], vtr[:])
                    nc.sync.dma_start(
                        vc_dram.rearrange("(t p) d -> p t d", p=128),
                        vsb_all[:])
                if groups and not skip_cc:
                    nc.gpsimd.collective_compute(
                        "AllGather", mybir.AluOpType.bypass,
                        replica_groups=groups,
                        ins=[vc_dram.opt()], outs=[vg_dram.opt()],
                    )
                for b in range(N_CORES if groups else 0):
                    nc.sync.dma_start(
                        v_g[b // KTG][:, (b % KTG) * NLB:(b % KTG + 1) * NLB, :],
                        vg_dram[b].rearrange("(t p) d -> p t d", p=128))

            if cut == "proj":
                nc.sync.dma_start(out[0:128, :], qt_sb[:, 0:128].bitcast(F32))
            if cut == "gather":
                nc.sync.dma_start(out[0:128, :], v_b[0][:, 0, :].bitcast(F32))

            # ---- phase B: attention over all 1024 queries at once,
            # one loop over the 64 key blocks, software-pipelined ----
            NJ = NKB if cut == "full" else 0
            with (
                tc.tile_pool(name="ps_st", bufs=3, space="PSUM") as ps_st,
                tc.tile_pool(name="ps_o", bufs=1, space="PSUM") as ps_o,
            ):
                o_ps = ps_o.tile([128, SL], F32, tag="o")
                acc = fin_pool.tile([128, SL], F32, tag="acc")
                acc_g = fin_pool.tile([128, SL], F32, tag="acc_g")
                # key blocks ordered sequence-half-first so attention can
                # start as soon as the first half-gather lands
                js = [(h, b, t) for h in range(2)
                      for b in range(N_CORES) for t in range(4)]
                pts = {}
                n_d = n_g = 0
                for j in range(NJ + 1):
                    if j < NJ:
                        h, b, t = js[j]
                        st_ps = ps_st.tile([128, SL], F32, tag="st")
                        for u in range(2):
                            nc.tensor.matmul(
                                st_ps[:, u * 512:(u + 1) * 512],
                                kth[h][:, b * 512 + t * 128:
                                        b * 512 + (t + 1) * 128],
                                qt_sb[:, u * 512:(u + 1) * 512],
                                start=True, stop=True,
                            )
                        pt = pt_pool.tile([128, SL], mm_dt, tag="pt")
                        nc.scalar.activation(
                            pt[:], st_ps[:], mybir.ActivationFunctionType.Exp)
                        pts[j] = pt
                    if j > 0:
                        pt = pts.pop(j - 1)
                        h, b, t = js[j - 1]
                        for u in range(2):
                            nc.tensor.matmul(
                                o_ps[:, u * 512:(u + 1) * 512],
                                v_b[b][:, h * 4 + t, :],
                                pt[:, u * 512:(u + 1) * 512],
                                start=(j - 1 == 0), stop=(j - 1 == NKB - 1))
                        if (j - 1) % 3 == 2:
                            if n_g == 0:
                                nc.gpsimd.tensor_copy(
                                    acc_g[:], pt[:].bitcast(F32))
                            else:
                                nc.gpsimd.tensor_add(
                                    acc_g[:], acc_g[:], pt[:].bitcast(F32))
                            n_g += 1
                        else:
                            if n_d == 0:
                                nc.vector.tensor_copy(acc[:], pt[:].bitcast(F32))
                            else:
                                nc.vector.tensor_add(
                                    acc[:], acc[:], pt[:].bitcast(F32))
                            n_d += 1

                if NJ:
                    # partition-reduce the exp accumulator, then normalize O^T
                    nc.vector.tensor_add(acc[:], acc[:], acc_g[:])
                    cs_ps = ps_st.tile([128, SL], F32, tag="st")
                    for u in range(2):
                        us = slice(u * 512, (u + 1) * 512)
                        nc.tensor.matmul(cs_ps[:, us], onesf[:], acc[:, us],
                                         start=True, stop=True)
                    o_sb = fin_pool.tile([128, SL], F32, tag="o_sb")
                    rcs = fin_pool.tile([128, SL], F32, tag="rcs")
                    oo_all = fin_pool.tile([128, NLB, DK], F32, tag="oo")
                    for u in range(2):
                        us = slice(u * 512, (u + 1) * 512)
                        nc.vector.reciprocal(rcs[:, us], cs_ps[:, us])
                        nc.vector.tensor_mul(o_sb[:, us], o_ps[:, us], rcs[:, us])
                        for t in range(u * 4, u * 4 + 4):
                            otr = ps_st.tile([128, 128], F32, tag="st")
                            nc.tensor.transpose(
                                otr[:], o_sb[:, t * 128:(t + 1) * 128],
                                ident[:].bitcast(F32))
                            nc.vector.tensor_copy(oo_all[:, t, :], otr[:])
                    nc.sync.dma_start(
                        out.rearrange("(t p) d -> p t d", p=128), oo_all[:])

    nc.compile()
    return nc


_NC_CACHE = {}


def _get_nc(mm_dt):
    cut = os.environ.get("KCUT", "full")
    key = (str(mm_dt), cut, os.environ.get("KCC", ""))
    if key not in _NC_CACHE:
        _NC_CACHE[key] = _build(mm_dt, cut)
    return _NC_CACHE[key]


def _run(inputs, trace=False, mm_dt=None, **spmd_kwargs):
    if mm_dt is None:
        mm_dt = (mybir.dt.float32r
                 if os.environ.get("KDT", "f32r") == "f32r" else F32)
    x = np.asarray(inputs["x"], dtype=np.float32)
    Wq = np.asarray(inputs["Wq"], dtype=np.float32)
    Wk = np.asarray(inputs["Wk"], dtype=np.float32)
    Wv = np.asarray(inputs["Wv"], dtype=np.float32)
    bq = np.asarray(inputs["bq"], dtype=np.float32)
    bk = np.asarray(inputs["bk"], dtype=np.float32)
    bv = np.asarray(inputs["bv"], dtype=np.float32)

    wqT = np.ascontiguousarray((Wq / SCALE).T)
    wkT = np.ascontiguousarray(Wk.T)
    wvT = np.ascontiguousarray(Wv.T)
    shared = {
        "wqT": wqT, "wkT": wkT, "wvT": wvT,
        "b_pack": np.ascontiguousarray(
            np.stack([bq / SCALE, bk, bv], axis=1).astype(np.float32)),
        "ones_d": np.ones((128, 128), dtype=np.float32),
        "ident_d": np.eye(128, dtype=np.float32),
    }
    in_maps = []
    for c in range(N_CORES):
        xT_c = np.ascontiguousarray(x[c * SL:(c + 1) * SL].T)
        in_maps.append({"xT": xT_c, **shared})

    nc = _get_nc(mm_dt)
    res = run_bass_kernel_spmd(
        nc, in_maps, core_ids=list(range(N_CORES)), trace=trace, **spmd_kwargs)
    full = np.concatenate(
        [res.results[c]["out"] for c in range(N_CORES)], axis=0)
    return full, res


def kernel(**inputs):
    out, _ = _run(inputs)
    return out


# revision 41
# speedup vs baseline: 777.7829x; 1.0204x over previous
"""Single-head attention (SEQ=8192, D_MODEL=2048, D_K=128) on 8 TRN2 NeuronCores.

Sharding: sequence-parallel. Each core owns 1024 query rows. QKV projections
computed on the local shard; K^T and V all-gathered across the 8 cores
(d_k=128 lives on the partition dim, so gathered blocks DMA straight into
matmul operand layouts with no device-side transposes). Attention runs in
S^T layout ([key, query] tiles): exp on the scalar engine, P@V on the tensor
engine (software-pipelined one block behind the scores), and the softmax
denominator accumulated on the vector engine.
"""
import os

import numpy as np

import concourse.bacc as bacc
import concourse.tile as tile
from concourse import mybir
from concourse.bass_utils import run_bass_kernel_spmd

N_CORES = 8
SEQ = 8192
DM = 2048
DK = 128
SL = SEQ // N_CORES          # 1024 local rows
NMC = DM // 128              # 16 contraction chunks for projections
NKB = SEQ // 128             # 64 key blocks
NLB = SL // 128              # 8 key blocks per gathered core-block
SCALE = float(np.sqrt(DK))

F32 = mybir.dt.float32


def _build(mm_dt, cut="full"):
    nc = bacc.Bacc(
        "TRN2",
        target_bir_lowering=False,
        debug=False,
        num_devices=N_CORES,
    )

    xT = nc.dram_tensor("xT", [DM, SL], mm_dt, kind="ExternalInput")
    wqT = nc.dram_tensor("wqT", [DM, DK], mm_dt, kind="ExternalInput")
    wkT = nc.dram_tensor("wkT", [DM, DK], mm_dt, kind="ExternalInput")
    wvT = nc.dram_tensor("wvT", [DM, DK], mm_dt, kind="ExternalInput")
    ones_d = nc.dram_tensor("ones_d", [128, 128], mm_dt, kind="ExternalInput")
    ident_d = nc.dram_tensor("ident_d", [128, 128], mm_dt, kind="ExternalInput")
    b_pack = nc.dram_tensor("b_pack", [DK, 3], F32, kind="ExternalInput")
    out = nc.dram_tensor("out", [SL, DK], F32, kind="ExternalOutput")

    CP = mybir.ActivationFunctionType.Copy
    ID = mybir.ActivationFunctionType.Identity

    with tile.TileContext(nc) as tc:
        with (
            tc.tile_pool(name="const", bufs=1) as const_pool,
            tc.tile_pool(name="proj", bufs=1) as proj_pool,
            tc.tile_pool(name="kv", bufs=1) as kv_pool,
            tc.tile_pool(name="w", bufs=1) as w_pool,
            tc.tile_pool(name="vs", bufs=4) as vs_pool,
            tc.tile_pool(name="pt", bufs=4) as pt_pool,
            tc.tile_pool(name="fin", bufs=1) as fin_pool,
            tc.tile_pool(name="dram", bufs=1, space="DRAM") as dram_pool,
        ):
            # ---- constants (loads emitted after the first x group) ----
            ident = const_pool.tile([128, 128], mm_dt)
            onesf = const_pool.tile([128, 128], F32)
            ones_r = const_pool.tile([128, 128], mm_dt)
            b_sb = const_pool.tile([DK, 3], F32)
            bq_sb, bk_sb, bv_sb = (b_sb[:, 0:1], b_sb[:, 1:2], b_sb[:, 2:3])

            # ---- inputs loaded per-chunk (first matmuls start after ~2 DMAs)
            wk_t = w_pool.tile([128, NMC, DK], mm_dt)
            wq_t = w_pool.tile([128, NMC, DK], mm_dt)
            wv_t = w_pool.tile([128, NMC, DK], mm_dt)
            half = (NMC // 2) * 128
            nc.sync.dma_start(
                wk_t[:, :NMC // 2, :],
                wkT[:half, :].rearrange("(c p) d -> p c d", p=128))
            wk_sb = [wk_t[:, i, :] for i in range(NMC)]
            wq_sb = [wq_t[:, i, :] for i in range(NMC)]
            wv_sb = [wv_t[:, i, :] for i in range(NMC)]
            # x loaded in column halves (local sequence halves), half 0
            # first, so the K projection for half 0 — and its all-gather —
            # start as early as possible.
            XG = 2                       # x chunks per load group
            NG = NMC // XG
            x_hg = [[w_pool.tile([128, XG, 512], mm_dt, name=f"x{h}g{g}")
                     for g in range(NG)] for h in range(2)]

            def x_half(i, h):
                return x_hg[h][i // XG][:, i % XG, :]

            for h in range(2):
                for g in range(NG):
                    if h == 0 and g == 1:
                        nc.sync.dma_start(
                            wk_t[:, NMC // 2:, :],
                            wkT[half:, :].rearrange("(c p) d -> p c d", p=128))
                        nc.sync.dma_start(b_sb[:], b_pack[:])
                        nc.sync.dma_start(ones_r[:], ones_d[:])
                        nc.sync.dma_start(ident[:], ident_d[:])
                        nc.sync.dma_start(onesf[:], ones_d[:].bitcast(F32))
                    if h == 0 and g == 4:
                        nc.sync.dma_start(
                            wq_t[:], wqT.rearrange("(c p) d -> p c d", p=128))
                    if h == 1 and g == 0:
                        nc.sync.dma_start(
                            wv_t[:], wvT.rearrange("(c p) d -> p c d", p=128))
                    nc.sync.dma_start(
                        x_hg[h][g][:],
                        xT[g * XG * 128:(g + 1) * XG * 128,
                           h * 512:(h + 1) * 512].rearrange(
                            "(c p) s -> p c s", p=128))

            qt_sb = proj_pool.tile([128, SL], mm_dt)
            vt_loc = proj_pool.tile([128, SL], mm_dt)
            kth_loc = [proj_pool.tile([128, 512], mm_dt, name=f"ktl{h}")
                       for h in range(2)]
            ktd_h = [dram_pool.tile([128, 512], mm_dt, name=f"ktd{h}")
                     for h in range(2)]
            vc_dram = dram_pool.tile([SL, DK], mm_dt)

            ktg_h = [dram_pool.tile([N_CORES, 128, 512], mm_dt,
                                    addr_space="Shared", name=f"ktgd{h}")
                     for h in range(2)]
            vg_dram = dram_pool.tile(
                [N_CORES, SL, DK], mm_dt, addr_space="Shared")
            groups = [list(range(N_CORES))]
            if cut == "proj":
                groups = None  # sections below are skipped
            skip_cc = os.environ.get("KCC", "") == "skip"

            KTG = 4                      # gathered core-blocks per staging DMA
            # gathered K^T, one tile per sequence half: [128, 8 cores x 512]
            kth = [kv_pool.tile([128, N_CORES * 512], mm_dt, name=f"kth{h}")
                   for h in range(2)]
            v_g = [kv_pool.tile([128, KTG * NLB, DK], mm_dt, name=f"vg{g}")
                   for g in range(N_CORES // KTG)]
            v_b = [v_g[b // KTG][:, (b % KTG) * NLB:(b % KTG + 1) * NLB, :]
                   for b in range(N_CORES)]

            # ---- phase A: projections. K in sequence halves (each half's
            # all-gather launches as soon as that half of x has landed),
            # then Q, then V.
            with tc.tile_pool(name="ps_proj", bufs=2, space="PSUM") as ps_proj:
                for h in range(2):
                    kt_ps_h = ps_proj.tile([128, 512], F32, tag="projh")
                    for i in range(NMC):
                        nc.tensor.matmul(kt_ps_h[:], wk_sb[i], x_half(i, h),
                                         start=(i == 0), stop=(i == NMC - 1))
                    nc.scalar.activation(
                        kth_loc[h][:], kt_ps_h[:], ID, bias=bk_sb[:])
                    nc.sync.dma_start(ktd_h[h][:], kth_loc[h][:])
                    if groups and not skip_cc:
                        nc.gpsimd.collective_compute(
                            "AllGather", mybir.AluOpType.bypass,
                            replica_groups=groups,
                            ins=[ktd_h[h].opt()], outs=[ktg_h[h].opt()],
                        )
                    for b in range(N_CORES if groups else 0):
                        nc.sync.dma_start(
                            kth[h][:, b * 512:(b + 1) * 512], ktg_h[h][b])

                # Q projection (overlaps the K gathers)
                qt_ps = ps_proj.tile([128, SL], F32, tag="proj")
                for i in range(NMC):
                    for h in range(2):
                        hs = slice(h * 512, (h + 1) * 512)
                        nc.tensor.matmul(qt_ps[:, hs], wq_sb[i],
                                         x_half(i, h),
                                         start=(i == 0), stop=(i == NMC - 1))
                nc.scalar.activation(qt_sb[:], qt_ps[:], ID, bias=bq_sb[:])

                # V projection + PE transpose to natural layout
                vt_ps = ps_proj.tile([128, SL], F32, tag="proj")
                for i in range(NMC):
                    for h in range(2):
                        hs = slice(h * 512, (h + 1) * 512)
                        nc.tensor.matmul(vt_ps[:, hs], wv_sb[i],
                                         x_half(i, h),
                                         start=(i == 0), stop=(i == NMC - 1))
                nc.scalar.activation(vt_loc[:], vt_ps[:], ID, bias=bv_sb[:])
                vsb_all = vs_pool.tile([128, NLB, 128], mm_dt, tag="vsb",
                                       bufs=1)
                with tc.tile_pool(name="ps_vtr", bufs=2, space="PSUM") as ps_vtr:
                    for t in range(NLB):
                        vtr = ps_vtr.tile([128, 128], mm_dt, tag="vtr")
                        nc.tensor.transpose(
                            vtr[:], vt_loc[:, t * 128:(t + 1) * 128], ident[:])
                        nc.scalar.copy(vsb_all[:, t, :], vtr[:])
                    nc.sync.dma_start(
                        vc_dram.rearrange("(t p) d -> p t d", p=128),
                        vsb_all[:])
                if groups and not skip_cc:
                    nc.gpsimd.collective_compute(
                        "AllGather", mybir.AluOpType.bypass,
                        replica_groups=groups,
                        ins=[vc_dram.opt()], outs=[vg_dram.opt()],
                    )
                for b in range(N_CORES if groups else 0):
                    nc.sync.dma_start(
                        v_g[b // KTG][:, (b % KTG) * NLB:(b % KTG + 1) * NLB, :],
                        vg_dram[b].rearrange("(t p) d -> p t d", p=128))

            if cut == "proj":
                nc.sync.dma_start(out[0:128, :], qt_sb[:, 0:128].bitcast(F32))
            if cut == "gather":
                nc.sync.dma_start(out[0:128, :], v_b[0][:, 0, :].bitcast(F32))

            # ---- phase B: attention over all 1024 queries at once,
            # one loop over the 64 key blocks, software-pipelined ----
            NJ = NKB if cut == "full" else 0
            with (
                tc.tile_pool(name="ps_st", bufs=3, space="PSUM") as ps_st,
                tc.tile_pool(name="ps_o", bufs=1, space="PSUM") as ps_o,
            ):
                o_ps = ps_o.tile([128, SL], F32, tag="o")
                acc = fin_pool.tile([128, SL], mm_dt, tag="acc")
                acc_g = fin_pool.tile([128, SL], mm_dt, tag="acc_g")
                # key blocks ordered sequence-half-first so attention can
                # start as soon as the first half-gather lands
                js = [(h, b, t) for h in range(2)
                      for b in range(N_CORES) for t in range(4)]
                pts = {}
                tail_pts = []
                n_d = n_g = 0
                for j in range(NJ + 1):
                    if j < NJ:
                        h, b, t = js[j]
                        st_ps = ps_st.tile([128, SL], F32, tag="st")
                        for u in range(2):
                            nc.tensor.matmul(
                                st_ps[:, u * 512:(u + 1) * 512],
                                kth[h][:, b * 512 + t * 128:
                                        b * 512 + (t + 1) * 128],
                                qt_sb[:, u * 512:(u + 1) * 512],
                                start=True, stop=True,
                            )
                        pt = pt_pool.tile([128, SL], mm_dt, tag="pt")
                        nc.scalar.activation(
                            pt[:], st_ps[:], mybir.ActivationFunctionType.Exp)
                        pts[j] = pt
                    if j > 0:
                        pt = pts.pop(j - 1)
                        h, b, t = js[j - 1]
                        for u in range(2):
                            nc.tensor.matmul(
                                o_ps[:, u * 512:(u + 1) * 512],
                                v_b[b][:, h * 4 + t, :],
                                pt[:, u * 512:(u + 1) * 512],
                                start=(j - 1 == 0), stop=(j - 1 == NKB - 1))
                        if j - 1 >= NKB - 2:
                            tail_pts.append(pt)
                        elif (j - 1) % 3 == 2:
                            if n_g == 0:
                                nc.gpsimd.tensor_copy(acc_g[:], pt[:])
                            else:
                                nc.gpsimd.tensor_add(
                                    acc_g[:], acc_g[:], pt[:])
                            n_g += 1
                        else:
                            if n_d == 0:
                                nc.vector.tensor_copy(acc[:], pt[:])
                            else:
                                nc.vector.tensor_add(acc[:], acc[:], pt[:])
                            n_d += 1

                if NJ:
                    # partition-reduce all exp accumulators in one PSUM
                    # accumulation group (no serial DVE merge in the tail)
                    cs_ps = ps_st.tile([128, SL], F32, tag="st")
                    srcs = [acc[:], acc_g[:]] + [p[:] for p in tail_pts]
                    for u in range(2):
                        us = slice(u * 512, (u + 1) * 512)
                        for si, s_ap in enumerate(srcs):
                            nc.tensor.matmul(
                                cs_ps[:, us], ones_r[:], s_ap[:, us],
                                start=(si == 0), stop=(si == len(srcs) - 1))
                    rcs = fin_pool.tile([128, SL], F32, tag="rcs")
                    oo_all = fin_pool.tile([128, NLB, DK], F32, tag="oo")
                    for u in range(2):
                        us = slice(u * 512, (u + 1) * 512)
                        nc.vector.reciprocal(rcs[:, us], cs_ps[:, us])
                        nc.vector.tensor_mul(rcs[:, us], o_ps[:, us], rcs[:, us])
                        for t in range(u * 4, u * 4 + 4):
                            otr = ps_st.tile([128, 128], F32, tag="st")
                            nc.tensor.transpose(
                                otr[:], rcs[:, t * 128:(t + 1) * 128],
                                ident[:].bitcast(F32))
                            nc.vector.tensor_copy(oo_all[:, t, :], otr[:])
                    nc.sync.dma_start(
                        out.rearrange("(t p) d -> p t d", p=128), oo_all[:])

    nc.compile()
    return nc


_NC_CACHE = {}


def _get_nc(mm_dt):
    cut = os.environ.get("KCUT", "full")
    key = (str(mm_dt), cut, os.environ.get("KCC", ""))
    if key not in _NC_CACHE:
        _NC_CACHE[key] = _build(mm_dt, cut)
    return _NC_CACHE[key]


def _run(inputs, trace=False, mm_dt=None, **spmd_kwargs):
    if mm_dt is None:
        mm_dt = (mybir.dt.float32r
                 if os.environ.get("KDT", "f32r") == "f32r" else F32)
    x = np.asarray(inputs["x"], dtype=np.float32)
    Wq = np.asarray(inputs["Wq"], dtype=np.float32)
    Wk = np.asarray(inputs["Wk"], dtype=np.float32)
    Wv = np.asarray(inputs["Wv"], dtype=np.float32)
    bq = np.asarray(inputs["bq"], dtype=np.float32)
    bk = np.asarray(inputs["bk"], dtype=np.float32)
    bv = np.asarray(inputs["bv"], dtype=np.float32)

    wqT = np.ascontiguousarray((Wq / SCALE).T)
    wkT = np.ascontiguousarray(Wk.T)
    wvT = np.ascontiguousarray(Wv.T)
    shared = {
        "wqT": wqT, "wkT": wkT, "wvT": wvT,
        "b_pack": np.ascontiguousarray(
            np.stack([bq / SCALE, bk, bv], axis=1).astype(np.float32)),
        "ones_d": np.ones((128, 128), dtype=np.float32),
        "ident_d": np.eye(128, dtype=np.float32),
    }
    in_maps = []
    for c in range(N_CORES):
        xT_c = np.ascontiguousarray(x[c * SL:(c + 1) * SL].T)
        in_maps.append({"xT": xT_c, **shared})

    nc = _get_nc(mm_dt)
    res = run_bass_kernel_spmd(
        nc, in_maps, core_ids=list(range(N_CORES)), trace=trace, **spmd_kwargs)
    full = np.concatenate(
        [res.results[c]["out"] for c in range(N_CORES)], axis=0)
    return full, res


def kernel(**inputs):
    out, _ = _run(inputs)
    return out
